# revision 33
# baseline (speedup 1.0000x reference)
"""NeuralSort relaxed-permutation kernel for 8 Trainium2 NeuronCores.

out[b, i, j] = softmax_i( s_i * scaling_j - B_i ),  s = -scores[b]
  scaling_j = n - 1 - 2j   =>  z[i,j] = c_j * x_i - B_i  with x = scores[b],
  c_j = 2j + 1 - n,  B_i = sum_k |x_i - x_k|

Sharding: core c -> (batch b = c//2, j-half h = c%2). Each core emits the
full-i (n) by half-j (n/2) slab of batch b, j-major bf16; the host
transposes/upcasts/unpermutes while unsharding.

The i axis is presented to the device in rank-sorted order (host argsort of
the O(n) per-core prep; B then follows from prefix sums). In rank space each
softmax column j concentrates on ranks near j: outside a ~800-wide rank
window every exp underflows bf16 to exact 0 (z drops >60 below the column
max; dropped mass < n*e^-60). Each 128-j chunk therefore computes z only on
a static W=1536 window whose start the host picks per chunk, writes
exp(z - M')/D into cols [0, W) of its output row, zeros elsewhere, and the
host restores window offset + rank permutation. M'_j (the exp shift) is the
max of z over the 128 rank-bucket means, an underestimate of the true max
by < ~40 (host-computed in O(n)).

Device pipeline per 128-j chunk (j on partitions, window i on free):
  PE: z = c_j x_i - B_i, K=9 bf16 stacked matmul (exact via hi/mid/lo
      splits) into PSUM [128, W].
  ACT: e = exp(z - M') -> bf16 window tile, with accum_out row-sum -> D_j
      (single softmax pass: no second exp, no Ln, no table switches).
  DVE: e *= 1/D_j (per-partition reciprocal + scale).
  DMA: value window [128, W] write + zero tail [128, n-W] write per chunk.
      The tails are dependency-free DRAM->DRAM copies from a zeros input,
      issued ahead of / interleaved with the value DMAs so the DMA engines
      run gap-free from the first instruction: the kernel is DMA-bound at
      the 360 B/ns aggregate write bandwidth for its full 16 MiB output.
"""

from contextlib import ExitStack

import numpy as np
import ml_dtypes

import concourse.bass as bass
import concourse.tile as tile
from concourse import bacc, mybir
from concourse.bass_utils import run_bass_kernel_spmd

F32 = mybir.dt.float32
BF16 = mybir.dt.bfloat16
AF = mybir.ActivationFunctionType
ALU = mybir.AluOpType

N_CORES = 8
P = 128
W_DEFAULT = 1536  # static window width (empirical need ~800, 2x margin)
# fallback widths if an input's windows run wider than expected; the host
# picks the smallest compiled width that covers the measured span + margin
W_CHOICES = (1536, 2048, 4096)
THR = 60.0        # host window threshold on z below column max


def _bf(x):
    return np.asarray(x, dtype=ml_dtypes.bfloat16)


def _split3(x):
    x = np.asarray(x, dtype=np.float32)
    h = _bf(x)
    r = x - h.astype(np.float32)
    m = _bf(r)
    l = _bf(r - m.astype(np.float32))
    return h, m, l


def _split2(x):
    x = np.asarray(x, dtype=np.float32)
    h = _bf(x)
    l = _bf(x - h.astype(np.float32))
    return h, l


# K-row pairing for the z matmul (z = sum_k l9_row_k * r9_row_k):
# lhs rows from [chi, clo, ones]; rhs rows from [-Bh,-Bm,-Bl,xh,xm,xl].
_PAIRS = [
    (0, 3, 1.0),   # c_hi * x_h
    (2, 0, -1.0),  # 1 * -B_h
    (1, 3, 1.0),   # c_lo * x_h
    (0, 4, 1.0),   # c_hi * x_m
    (2, 1, -1.0),  # 1 * -B_m
    (1, 4, 1.0),   # c_lo * x_m
    (0, 5, 1.0),   # c_hi * x_l
    (2, 2, -1.0),  # 1 * -B_l
    (1, 5, 1.0),   # c_lo * x_l
]


def build_nc(n=4096, mode="pair", num_devices=N_CORES, w=W_DEFAULT):
    """mode "pair"/"timing": the per-core program is identical (no
    collectives); "timing" builds num_devices=1 for the cost model."""
    nj = n // 2
    njc = nj // P
    W = min(w, n)
    has_tail = W < n
    # PSUM is 16 KiB/partition; double-buffer z only when two tiles fit
    z_bufs = 2 if W <= 2048 else 1

    nc = bacc.Bacc(
        "TRN2", target_bir_lowering=False, debug=False, num_devices=num_devices
    )

    l9 = nc.dram_tensor("l9", [9, nj], BF16, kind="ExternalInput").ap()
    r9w = nc.dram_tensor("r9w", [9, njc * W], BF16, kind="ExternalInput").ap()
    nmcol = nc.dram_tensor("nmcol", [P, njc], F32, kind="ExternalInput").ap()
    if has_tail:
        zeros = nc.dram_tensor(
            "zeros", [P, n - W], BF16, kind="ExternalInput"
        ).ap()
    out = nc.dram_tensor("out", [nj, n], BF16, kind="ExternalOutput").ap()

    with tile.TileContext(nc) as tc, ExitStack() as ctx:
        cpool = ctx.enter_context(tc.tile_pool(name="consts", bufs=1))

        # Every row's [W, n) tail is zero, fed by dependency-free
        # DRAM->DRAM copies from a zeros input. Issued ahead of and
        # interleaved with the value DMAs, they keep the DMA engines
        # saturated from the first instruction while PE/ACT ramp up; the
        # in-order SP queue never parks a ready value DMA behind a long
        # run of pending zeros. Total DMA bytes are unchanged.
        def zdma(jc):
            if has_tail:
                nc.sync.dma_start(
                    out=out[jc * P : (jc + 1) * P, W:n], in_=zeros
                )

        zdma(0)
        r9w_s = cpool.tile([9, njc * W], BF16, tag="r9w")
        nc.sync.dma_start(out=r9w_s[:], in_=r9w)
        l9_s = cpool.tile([9, nj], BF16, tag="l9")
        nc.sync.dma_start(out=l9_s[:], in_=l9)
        nm_s = cpool.tile([P, njc], F32, tag="nmcol")
        nc.sync.dma_start(out=nm_s[:], in_=nmcol)
        nzh = min(6, njc)
        for jc in range(1, nzh):
            zdma(jc)

        spool = ctx.enter_context(
            tc.tile_pool(name="sz", bufs=z_bufs, space="PSUM")
        )
        mpool = ctx.enter_context(tc.tile_pool(name="m", bufs=8))
        outp = ctx.enter_context(tc.tile_pool(name="outp", bufs=6))

        for jc in range(njc):
            lhs = l9_s[:, jc * P : (jc + 1) * P]
            zp = spool.tile([P, W], F32, tag="sz")
            for o in range(0, W, 512):
                nc.tensor.matmul(
                    zp[:, o : o + 512],
                    lhs,
                    r9w_s[:, jc * W + o : jc * W + o + 512],
                    start=True,
                    stop=True,
                )
            ot = outp.tile([P, W], BF16, tag="ot")
            dacc = mpool.tile([P, 1], F32, tag="dacc")
            nc.scalar.activation(
                out=ot[:],
                in_=zp[:],
                func=AF.Exp,
                bias=nm_s[0:P, jc : jc + 1],
                scale=1.0,
                accum_out=dacc[:],
            )
            rec = mpool.tile([P, 1], F32, tag="rec")
            nc.vector.reciprocal(rec[:], dacc[:])
            nc.vector.tensor_scalar(
                out=ot[:],
                in0=ot[:],
                scalar1=rec[:, 0:1],
                scalar2=None,
                op0=ALU.mult,
            )
            nc.sync.dma_start(out=out[jc * P : (jc + 1) * P, 0:W], in_=ot[:])
            if nzh + jc < njc:
                zdma(nzh + jc)

    nc.compile()
    return nc


# ---------------------------------------------------------------------------


def make_in_maps(scores, n, w=W_DEFAULT):
    """Per-core inputs + per-core (order, window starts) for unsharding.
    Returns (in_maps, metas, covered); covered=False if some chunk's
    active span does not fit in w (caller retries with a wider build)."""
    W = min(w, n)
    nj = n // 2
    njc = nj // P
    bs = n // P
    c_full = (2 * np.arange(n) + 1 - n).astype(np.float64)
    ones_nj = np.ones(nj, np.float32)

    covered = True
    in_maps, metas = [], []
    cache = {}
    for c in range(N_CORES):
        bb, h = divmod(c, 2)
        if bb not in cache:
            x = np.asarray(scores[bb], np.float64)
            order = np.argsort(x, kind="stable")
            xs = x[order]
            S = xs.sum()
            cs = np.cumsum(xs)
            r = np.arange(n, dtype=np.float64)
            # B over sorted ranks via prefix sums:
            # sum_{k<r}(x_r-x_k) + sum_{k>r}(x_k-x_r)
            Bs = xs * r - (cs - xs) + (S - cs) - xs * (n - 1 - r)
            xbar = xs.reshape(P, bs).mean(1)
            Bbar = Bs.reshape(P, bs).mean(1)
            xh, xm, xl = _split3(xs.astype(np.float32))
            Bh, Bm, Bl = _split3(Bs.astype(np.float32))
            cache[bb] = (order, xs, Bs, xbar, Bbar, [Bh, Bm, Bl, xh, xm, xl])
        order, xs, Bs, xbar, Bbar, src = cache[bb]

        cj = c_full[h * nj : (h + 1) * nj]
        # exp shift: M'_j = max_bucket (c_j*xbar - Bbar), <= true col max
        Mp = (cj[:, None] * xbar[None, :] - Bbar[None, :]).max(1)
        nmcol = np.ascontiguousarray(
            (-Mp).astype(np.float32).reshape(njc, P).T
        )

        # per-chunk window starts from subsampled exact columns
        starts = np.empty(njc, np.int64)
        for jc in range(njc):
            jsub = np.arange(jc * P, (jc + 1) * P, 8)
            zsub = cj[jsub][:, None] * xs[None, :] - Bs[None, :]
            m = zsub.max(1)
            act = zsub > (m[:, None] - THR)
            first = int(act.argmax(1).min())
            last = int((n - 1 - act[:, ::-1].argmax(1)).max())
            o = int(np.clip((first + last) // 2 - W // 2, 0, n - W))
            # recenter if the subsample span busts the window (never seen)
            if first < o or last >= o + W:
                o = int(np.clip(first - (W - (last - first + 1)) // 2, 0, n - W))
            if first < o or last >= o + W or last - first + 1 > W - 128:
                covered = False
            starts[jc] = o

        r9w = np.zeros((9, njc * W), dtype=ml_dtypes.bfloat16)
        for jc in range(njc):
            sl = slice(starts[jc], starts[jc] + W)
            for k, (ls, rs, w) in enumerate(_PAIRS):
                r9w[k, jc * W : (jc + 1) * W] = (
                    src[rs][sl] if w > 0 else -src[rs][sl]
                )

        ch, cl = _split2(cj.astype(np.float32))
        lsrc = [ch, cl, ones_nj]
        l9 = np.zeros((9, nj), dtype=ml_dtypes.bfloat16)
        for k, (ls, rs, w) in enumerate(_PAIRS):
            l9[k] = lsrc[ls]

        im = {"l9": l9, "r9w": r9w, "nmcol": nmcol}
        if W < n:
            im["zeros"] = np.zeros((P, n - W), dtype=ml_dtypes.bfloat16)
        in_maps.append(im)
        metas.append((order, starts))
    return in_maps, metas, covered


_NC_CACHE = {}


def _get_nc(n, w):
    if (n, w) not in _NC_CACHE:
        _NC_CACHE[(n, w)] = build_nc(
            n=n, mode="pair", num_devices=N_CORES, w=w
        )
    return _NC_CACHE[(n, w)]


def kernel(scores):
    scores = np.asarray(scores, dtype=np.float32)
    b, n = scores.shape
    nj = n // 2
    njc = nj // P
    for w in W_CHOICES:
        w = min(w, n)
        in_maps, metas, covered = make_in_maps(scores, n, w)
        if covered or w >= n:
            break
    nc = _get_nc(n, w)
    res = run_bass_kernel_spmd(nc, in_maps, list(range(N_CORES)))
    out = np.zeros((b, n, n), dtype=np.float32)
    for c in range(N_CORES):
        bb, h = divmod(c, 2)
        order, starts = metas[c]
        dev = np.asarray(res.results[c]["out"]).astype(np.float32)  # [nj, n]
        tmp = np.zeros((nj, n), dtype=np.float32)
        for jc in range(njc):
            o = int(starts[jc])
            tmp[jc * P : (jc + 1) * P, o : o + w] = dev[
                jc * P : (jc + 1) * P, 0:w
            ]
        # out[bb, i, h*nj + jj] = tmp[jj, rank(i)]
        out[bb][order, h * nj : (h + 1) * nj] = tmp.T
    return out
